# revision 2
# baseline (speedup 1.0000x reference)
"""Distributed GPT-2 causal attention block for 8 TRN2 NeuronCores.

Sharding: data-parallel over batch (B=2) x tensor-parallel over heads
(16 heads -> 4 groups of 4). core = b*4 + g handles batch b, heads 4g..4g+3.

Per-core kernel (all compute in bf16, f32 PSUM accumulation):
  qT/kT = W[q|k]^T x^T        [2 tiles of 128 = 2 heads each, layout (h d) x S]
  v     = x W_v               [S x (4 heads x 65)], col 64 of each head = ones
  sT    = kT^T qT (transposed scores, row-packed 2 heads/matmul via tile rows)
  PT    = exp(sT/8) (ScalarE), causal: diag block masked multiplicatively,
          blocks above diag never computed
  av    = v_aug^T PT accumulated over key blocks -> [65 x S] PSUM;
          rows 0:64 unnormalized attn out^T, row 64 = softmax denominators r
  rinv  = 1/r via DRAM reshape roundtrip, broadcast to 64 partitions
  attT  = av[0:64] * rinv     [(h d) x S]
  outT  = wp^T attT           [NX x S] partial (sum over head groups on host)

Host: shard/cast inputs, run SPMD on cores 0-7, transpose+reduce partials,
fold b_attn's v-bias and b_proj in on the host (exact: softmax rows sum to 1).
"""

import numpy as np
import ml_dtypes

B, S, NX = 2, 2048, 1024
H, D = 16, 64
HPC = 4        # heads per core
KCH = NX // 128  # 8 contraction chunks
SQT = S // 128   # 16 query tiles
SCALE = 0.125    # 1/sqrt(D)

_nc_cache = None


def _av_chunks(j):
    """Bank-aligned (start, width) chunks of [j*128, S) on the 512-f32 grid."""
    out = []
    s = j * 128
    while s < S:
        e = min((s // 512 + 1) * 512, S)
        out.append((s, e - s))
        s = e
    return out


def _emit(nc, tc, bass, mybir, tens):
    dt = mybir.dt
    F32, BF16 = dt.float32, dt.bfloat16
    MULT = mybir.AluOpType.mult
    EXP = mybir.ActivationFunctionType.Exp
    xT, wq, wk, wv, wp, bq, bk, maskT, outT, r_dram, ri_dram = tens

    import contextlib
    with contextlib.ExitStack() as ctx:
        consts = ctx.enter_context(tc.tile_pool(name="consts", bufs=1))
        wpool = ctx.enter_context(tc.tile_pool(name="wpool", bufs=1))
        xpool = ctx.enter_context(tc.tile_pool(name="xpool", bufs=1))
        qkpool = ctx.enter_context(tc.tile_pool(name="qkpool", bufs=1))
        vpool = ctx.enter_context(tc.tile_pool(name="vpool", bufs=1))
        ptpool = ctx.enter_context(tc.tile_pool(name="ptpool", bufs=1))
        atpool = ctx.enter_context(tc.tile_pool(name="atpool", bufs=1))
        rpool = ctx.enter_context(tc.tile_pool(name="rpool", bufs=2))
        opool = ctx.enter_context(tc.tile_pool(name="opool", bufs=2))
        psp = ctx.enter_context(tc.tile_pool(name="psp", bufs=2, space="PSUM"))
        avp = ctx.enter_context(tc.tile_pool(name="avp", bufs=1, space="PSUM"))

        # ---- constants & weights ----
        mask_sb = consts.tile([128, 128], BF16, tag="mask")
        nc.sync.dma_start(out=mask_sb[:, :], in_=maskT.ap())
        bq_sb = consts.tile([128, 2], F32, tag="bq")
        nc.sync.dma_start(out=bq_sb[:, :],
                          in_=bq.ap().rearrange("(t p) o -> p (t o)", p=128))
        bk_sb = consts.tile([128, 2], F32, tag="bk")
        nc.sync.dma_start(out=bk_sb[:, :],
                          in_=bk.ap().rearrange("(t p) o -> p (t o)", p=128))

        wq_sb = wpool.tile([128, KCH, HPC * D], BF16, tag="wq")
        nc.sync.dma_start(out=wq_sb[:], in_=wq.ap().rearrange("(k p) n -> p k n", p=128))
        wk_sb = wpool.tile([128, KCH, HPC * D], BF16, tag="wk")
        nc.sync.dma_start(out=wk_sb[:], in_=wk.ap().rearrange("(k p) n -> p k n", p=128))
        wv_sb = wpool.tile([128, KCH, HPC * D], BF16, tag="wv")
        nc.sync.dma_start(out=wv_sb[:], in_=wv.ap().rearrange("(k p) n -> p k n", p=128))
        wp_sb = wpool.tile([128, 2, NX], BF16, tag="wp")
        nc.sync.dma_start(out=wp_sb[:], in_=wp.ap().rearrange("(k p) n -> p k n", p=128))

        x_sb = []
        for k in range(KCH):
            t = xpool.tile([128, S], BF16, tag=f"x{k}")
            nc.sync.dma_start(out=t[:, :], in_=xT.ap()[k * 128:(k + 1) * 128, :])
            x_sb.append(t)

        # ---- qT / kT : [128 (2 heads x 64), S] bf16, + bias ----
        qt_sb, kt_sb = [], []
        for t in range(2):
            qt_sb.append(qkpool.tile([128, S], BF16, tag=f"qt{t}", name=f"qt{t}"))
            kt_sb.append(qkpool.tile([128, S], BF16, tag=f"kt{t}", name=f"kt{t}"))
        for (dst, wsb, bsb) in ((qt_sb, wq_sb, bq_sb), (kt_sb, wk_sb, bk_sb)):
            for t in range(2):
                for c in range(S // 512):
                    ps = psp.tile([128, 512], F32, tag="sps")
                    for k in range(KCH):
                        nc.tensor.matmul(
                            ps[:, :],
                            lhsT=wsb[:, k, t * 128:(t + 1) * 128],
                            rhs=x_sb[k][:, c * 512:(c + 1) * 512],
                            start=(k == 0), stop=(k == KCH - 1))
                    nc.vector.tensor_scalar_add(
                        out=dst[t][:, c * 512:(c + 1) * 512], in0=ps[:, :],
                        scalar1=bsb[:, t:t + 1])

        # ---- v_aug : per key-tile [128, 4*65], col 64 of each head = 1.0 ----
        v_sb = []
        for j in range(SQT):
            vt = vpool.tile([128, HPC * 65], BF16, tag=f"v{j}", name=f"v{j}")
            v_sb.append(vt)
            vt3 = vt.rearrange("p (h e) -> p h e", e=65)
            nc.gpsimd.memset(vt3[:, :, 64:65], 1.0)
            ps = psp.tile([128, 512], F32, tag="sps")
            for k in range(KCH):
                nc.tensor.matmul(
                    ps[:, 0:HPC * D],
                    lhsT=x_sb[k][:, j * 128:(j + 1) * 128],
                    rhs=wv_sb[:, k, :],
                    start=(k == 0), stop=(k == KCH - 1))
            nc.vector.tensor_copy(
                out=vt3[:, :, 0:64],
                in_=ps[:, 0:HPC * D].rearrange("p (h d) -> p h d", d=64))

        # ---- attention, one head-pair at a time ----
        attT = [atpool.tile([128, S], BF16, tag=f"attT{hp}", name=f"attT{hp}")
                 for hp in range(2)]
        for hp in range(2):
            # scores^T + exp -> PT tiles
            pt = [[None] * SQT, [None] * SQT]
            for j in range(SQT):
                W = S - j * 128
                for hl in range(2):
                    pt[hl][j] = ptpool.tile([128, W], BF16, tag=f"pt{hl}_{j}",
                                            name=f"pt{hl}_{j}_{hp}")
                for c in range((W + 1023) // 1024):
                    w = min(1024, W - c * 1024)
                    o = j * 128 + c * 1024
                    for hl in range(2):
                        sps = psp.tile([128, 1024], F32, tag="sps")
                        for cc in range((w + 511) // 512):
                            ww = min(512, w - cc * 512)
                            nc.tensor.matmul(
                                sps[:, cc * 512:cc * 512 + ww],
                                lhsT=kt_sb[hp][hl * 64:(hl + 1) * 64,
                                               j * 128:(j + 1) * 128],
                                rhs=qt_sb[hp][hl * 64:(hl + 1) * 64,
                                              o + cc * 512:o + cc * 512 + ww],
                                start=True, stop=True)
                        nc.scalar.activation(
                            out=pt[hl][j][:, c * 1024:c * 1024 + w],
                            in_=sps[:, :w], func=EXP, scale=SCALE)
                for hl in range(2):
                    nc.vector.tensor_tensor(
                        out=pt[hl][j][:, 0:128], in0=pt[hl][j][:, 0:128],
                        in1=mask_sb[:, :], op=MULT)

            # AV + normalization per head
            for hl in range(2):
                h = hp * 2 + hl
                av = avp.tile([65, S], F32, tag="av")
                for j in range(SQT):
                    for (cs, cw) in _av_chunks(j):
                        nc.tensor.matmul(
                            av[:, cs:cs + cw],
                            lhsT=v_sb[j][:, h * 65:(h + 1) * 65],
                            rhs=pt[hl][j][:, cs - j * 128:cs - j * 128 + cw],
                            start=(j == 0), stop=(j == SQT - 1),
                            skip_group_check=True)
                # r -> DRAM -> [128,16] -> 1/r -> DRAM -> broadcast [64, S]
                rt = rpool.tile([128, S], F32, tag="rt")
                nc.vector.tensor_copy(out=rt[64:65, :], in_=av[64:65, :])
                nc.gpsimd.dma_start(out=r_dram.ap()[h:h + 1, :], in_=rt[64:65, :])
                rsh = rpool.tile([128, 16], F32, tag="rsh")
                nc.gpsimd.dma_start(
                    out=rsh[:, :],
                    in_=bass.AP(tensor=r_dram, offset=h * S, ap=[[1, 128], [128, 16]]))
                rsi = rpool.tile([128, 16], F32, tag="rsi")
                nc.vector.reciprocal(out=rsi[:, :], in_=rsh[:, :])
                nc.gpsimd.dma_start(
                    out=bass.AP(tensor=ri_dram, offset=h * S, ap=[[1, 128], [128, 16]]),
                    in_=rsi[:, :])
                nc.gpsimd.dma_start(
                    out=rt[0:64, :],
                    in_=bass.AP(tensor=ri_dram, offset=h * S, ap=[[0, 64], [1, S]]))
                if hl == 0:
                    nc.vector.tensor_tensor(
                        out=attT[hp][0:64, :], in0=av[0:64, :], in1=rt[0:64, :],
                        op=MULT)
                else:
                    ats = rpool.tile([64, S], BF16, tag="ats")
                    nc.vector.tensor_tensor(
                        out=ats[:, :], in0=av[0:64, :], in1=rt[0:64, :], op=MULT)
                    nc.sync.dma_start(out=attT[hp][64:128, :], in_=ats[:, :])

        # ---- projection: outT[n, s] = sum_k wp[k, n] attT[k, s] ----
        for n in range(NX // 128):
            osb = opool.tile([128, S], F32, tag="osb")
            for sc in range(S // 512):
                ps = psp.tile([128, 512], F32, tag="sps")
                for kk in range(2):
                    nc.tensor.matmul(
                        ps[:, :],
                        lhsT=wp_sb[:, kk, n * 128:(n + 1) * 128],
                        rhs=attT[kk][:, sc * 512:(sc + 1) * 512],
                        start=(kk == 0), stop=(kk == 1))
                nc.vector.tensor_copy(out=osb[:, sc * 512:(sc + 1) * 512], in_=ps[:, :])
            nc.sync.dma_start(out=outT.ap()[n * 128:(n + 1) * 128, :], in_=osb[:, :])


def build_nc():
    import concourse.bass as bass
    import concourse.mybir as mybir
    import concourse.tile as tile
    from concourse import bacc
    dt = mybir.dt
    F32, BF16 = dt.float32, dt.bfloat16

    nc = bacc.Bacc("TRN2", target_bir_lowering=False, debug=False, num_devices=8)
    xT = nc.dram_tensor("xT", [NX, S], BF16, kind="ExternalInput")
    wq = nc.dram_tensor("wq", [NX, HPC * D], BF16, kind="ExternalInput")
    wk = nc.dram_tensor("wk", [NX, HPC * D], BF16, kind="ExternalInput")
    wv = nc.dram_tensor("wv", [NX, HPC * D], BF16, kind="ExternalInput")
    wp = nc.dram_tensor("wp", [HPC * D, NX], BF16, kind="ExternalInput")
    bq = nc.dram_tensor("bq", [HPC * D, 1], F32, kind="ExternalInput")
    bk = nc.dram_tensor("bk", [HPC * D, 1], F32, kind="ExternalInput")
    maskT = nc.dram_tensor("maskT", [128, 128], BF16, kind="ExternalInput")
    outT = nc.dram_tensor("outT", [NX, S], F32, kind="ExternalOutput")
    r_dram = nc.dram_tensor("r_scr", [HPC, S], F32)
    ri_dram = nc.dram_tensor("ri_scr", [HPC, S], F32)
    tens = (xT, wq, wk, wv, wp, bq, bk, maskT, outT, r_dram, ri_dram)

    with tile.TileContext(nc) as tc:
        _emit(nc, tc, bass, mybir, tens)
    nc.compile()
    return nc


def make_in_maps(x, w_attn, b_attn):
    bf = ml_dtypes.bfloat16
    maskT = np.triu(np.ones((128, 128), np.float32)).astype(bf)
    in_maps = []
    for core in range(8):
        b, g = divmod(core, 4)
        qs, ks, vs = 256 * g, NX + 256 * g, 2 * NX + 256 * g
        in_maps.append({
            "xT": np.ascontiguousarray(x[b].T).astype(bf),
            "wq": np.ascontiguousarray(w_attn[:, qs:qs + 256]).astype(bf),
            "wk": np.ascontiguousarray(w_attn[:, ks:ks + 256]).astype(bf),
            "wv": np.ascontiguousarray(w_attn[:, vs:vs + 256]).astype(bf),
            "wp": None,  # filled below (needs w_proj)
            "bq": b_attn[qs:qs + 256].reshape(256, 1).astype(np.float32),
            "bk": b_attn[ks:ks + 256].reshape(256, 1).astype(np.float32),
            "maskT": maskT,
        })
    return in_maps


def kernel(**inputs):
    global _nc_cache
    x = np.asarray(inputs["x"], np.float32)
    w_attn = np.asarray(inputs["w_attn"], np.float32)
    b_attn = np.asarray(inputs["b_attn"], np.float32)
    w_proj = np.asarray(inputs["w_proj"], np.float32)
    b_proj = np.asarray(inputs["b_proj"], np.float32)

    bf = ml_dtypes.bfloat16
    in_maps = make_in_maps(x, w_attn, b_attn)
    for core in range(8):
        g = core % 4
        in_maps[core]["wp"] = np.ascontiguousarray(
            w_proj[256 * g:256 * (g + 1), :]).astype(bf)

    if _nc_cache is None:
        _nc_cache = build_nc()
    from concourse.bass_utils import run_bass_kernel_spmd
    res = run_bass_kernel_spmd(_nc_cache, in_maps, core_ids=list(range(8)))

    out = np.zeros((B, S, NX), np.float32)
    for core in range(8):
        out[core // 4] += res.results[core]["outT"].T
    bv = b_attn[2 * NX:3 * NX]
    out += (bv @ w_proj + b_proj)[None, None, :]
    return out


# revision 5
# speedup vs baseline: 1.2158x; 1.2158x over previous
"""Distributed GPT-2 causal attention block for 8 TRN2 NeuronCores.

Sharding: data-parallel over batch (B=2) x tensor-parallel over heads
(16 heads -> 4 groups of 4). core = b*4 + g handles batch b, heads 4g..4g+3.

Per-core kernel (all compute in bf16, f32 PSUM accumulation):
  qT/kT = W[q|k]^T x^T        [2 tiles of 128 = 2 heads each, layout (h d) x S]
  v     = x W_v               [S x (4 heads x 65)], col 64 of each head = ones
  sT    = kT^T qT (transposed scores, row-packed 2 heads/matmul via tile rows)
  PT    = exp(sT/8) (ScalarE), causal: diag block masked multiplicatively,
          blocks above diag never computed
  av    = v_aug^T PT accumulated over key blocks -> [65 x S] PSUM;
          rows 0:64 unnormalized attn out^T, row 64 = softmax denominators r
  rinv  = 1/r via DRAM reshape roundtrip, broadcast to 64 partitions
  attT  = av[0:64] * rinv     [(h d) x S]
  outT  = wp^T attT           [NX x S] partial (sum over head groups on host)

Host: shard/cast inputs, run SPMD on cores 0-7, transpose+reduce partials,
fold b_attn's v-bias and b_proj in on the host (exact: softmax rows sum to 1).
"""

import numpy as np
import ml_dtypes

B, S, NX = 2, 2048, 1024
H, D = 16, 64
HPC = 4        # heads per core
KCH = NX // 128  # 8 contraction chunks
SQT = S // 128   # 16 query tiles
SCALE = 0.125    # 1/sqrt(D)

_nc_cache = None


def _av_chunks(j):
    """Bank-aligned (start, width) chunks of [j*128, S) on the 512-f32 grid."""
    out = []
    s = j * 128
    while s < S:
        e = min((s // 512 + 1) * 512, S)
        out.append((s, e - s))
        s = e
    return out


def _emit(nc, tc, bass, mybir, tens):
    dt = mybir.dt
    F32, BF16 = dt.float32, dt.bfloat16
    MULT = mybir.AluOpType.mult
    EXP = mybir.ActivationFunctionType.Exp
    xT, wq, wk, wv, wp, bq, bk, maskT, outT, r_dram, ri_dram = tens

    import contextlib
    with contextlib.ExitStack() as ctx:
        consts = ctx.enter_context(tc.tile_pool(name="consts", bufs=1))
        wpool = ctx.enter_context(tc.tile_pool(name="wpool", bufs=1))
        xpool = ctx.enter_context(tc.tile_pool(name="xpool", bufs=1))
        qkpool = ctx.enter_context(tc.tile_pool(name="qkpool", bufs=1))
        vpool = ctx.enter_context(tc.tile_pool(name="vpool", bufs=1))
        ptpool = ctx.enter_context(tc.tile_pool(name="ptpool", bufs=1))
        atpool = ctx.enter_context(tc.tile_pool(name="atpool", bufs=1))
        rpool = ctx.enter_context(tc.tile_pool(name="rpool", bufs=2))
        opool = ctx.enter_context(tc.tile_pool(name="opool", bufs=2))
        psp = ctx.enter_context(tc.tile_pool(name="psp", bufs=2, space="PSUM"))
        avp = ctx.enter_context(tc.tile_pool(name="avp", bufs=1, space="PSUM"))

        # ---- constants & weights ----
        mask_sb = consts.tile([128, 128], BF16, tag="mask")
        nc.sync.dma_start(out=mask_sb[:, :], in_=maskT.ap())
        bq_sb = consts.tile([128, 2], F32, tag="bq")
        nc.sync.dma_start(out=bq_sb[:, :],
                          in_=bq.ap().rearrange("(t p) o -> p (t o)", p=128))
        bk_sb = consts.tile([128, 2], F32, tag="bk")
        nc.sync.dma_start(out=bk_sb[:, :],
                          in_=bk.ap().rearrange("(t p) o -> p (t o)", p=128))

        wq_sb = wpool.tile([128, KCH, HPC * D], BF16, tag="wq")
        nc.sync.dma_start(out=wq_sb[:], in_=wq.ap().rearrange("(k p) n -> p k n", p=128))
        wk_sb = wpool.tile([128, KCH, HPC * D], BF16, tag="wk")
        nc.sync.dma_start(out=wk_sb[:], in_=wk.ap().rearrange("(k p) n -> p k n", p=128))
        wv_sb = wpool.tile([128, KCH, HPC * D], BF16, tag="wv")
        nc.sync.dma_start(out=wv_sb[:], in_=wv.ap().rearrange("(k p) n -> p k n", p=128))
        wp_sb = wpool.tile([128, 2, NX], BF16, tag="wp")
        nc.sync.dma_start(out=wp_sb[:], in_=wp.ap().rearrange("(k p) n -> p k n", p=128))

        x_sb = []
        for k in range(KCH):
            t = xpool.tile([128, S], BF16, tag=f"x{k}", name=f"x{k}")
            nc.sync.dma_start(out=t[:, :], in_=xT.ap()[k * 128:(k + 1) * 128, :])
            x_sb.append(t)

        qt_sb, kt_sb = [], []
        for t in range(2):
            qt_sb.append(qkpool.tile([128, S], BF16, tag=f"qt{t}", name=f"qt{t}"))
            kt_sb.append(qkpool.tile([128, S], BF16, tag=f"kt{t}", name=f"kt{t}"))
        v_sb = [vpool.tile([128, HPC * 65], BF16, tag=f"v{j}", name=f"v{j}")
                for j in range(SQT)]
        attT = [atpool.tile([128, S], BF16, tag=f"attT{hp}", name=f"attT{hp}")
                for hp in range(2)]

        def emit_qk(t):
            # qT/kT tile t: heads 2t, 2t+1 in rows (h d); + bias, bf16
            for (dst, wsb, bsb) in ((qt_sb, wq_sb, bq_sb), (kt_sb, wk_sb, bk_sb)):
                for c in range(S // 512):
                    ps = psp.tile([128, 512], F32, tag="sps")
                    for k in range(KCH):
                        nc.tensor.matmul(
                            ps[:, :],
                            lhsT=wsb[:, k, t * 128:(t + 1) * 128],
                            rhs=x_sb[k][:, c * 512:(c + 1) * 512],
                            start=(k == 0), stop=(k == KCH - 1))
                    nc.vector.tensor_scalar_add(
                        out=dst[t][:, c * 512:(c + 1) * 512], in0=ps[:, :],
                        scalar1=bsb[:, t:t + 1])

        def emit_v():
            for j in range(SQT):
                vt3 = v_sb[j].rearrange("p (h e) -> p h e", e=65)
                nc.gpsimd.memset(vt3[:, :, 64:65], 1.0)
                ps = psp.tile([128, 512], F32, tag="sps")
                for k in range(KCH):
                    nc.tensor.matmul(
                        ps[:, 0:HPC * D],
                        lhsT=x_sb[k][:, j * 128:(j + 1) * 128],
                        rhs=wv_sb[:, k, :],
                        start=(k == 0), stop=(k == KCH - 1))
                nc.vector.tensor_copy(
                    out=vt3[:, :, 0:64],
                    in_=ps[:, 0:HPC * D].rearrange("p (h d) -> p h d", d=64))

        def emit_scores(h, j, pt_t):
            # scores^T [key block j x queries j*128..S] -> exp -> PT, bf16
            hp, hl = divmod(h, 2)
            W = S - j * 128
            for c in range((W + 1023) // 1024):
                w = min(1024, W - c * 1024)
                o = j * 128 + c * 1024
                sps = psp.tile([128, 1024], F32, tag="sps")
                for cc in range((w + 511) // 512):
                    ww = min(512, w - cc * 512)
                    nc.tensor.matmul(
                        sps[:, cc * 512:cc * 512 + ww],
                        lhsT=kt_sb[hp][hl * 64:(hl + 1) * 64,
                                       j * 128:(j + 1) * 128],
                        rhs=qt_sb[hp][hl * 64:(hl + 1) * 64,
                                      o + cc * 512:o + cc * 512 + ww],
                        start=True, stop=True)
                nc.scalar.activation(
                    out=pt_t[:, c * 1024:c * 1024 + w],
                    in_=sps[:, :w], func=EXP, scale=SCALE)
            nc.vector.tensor_tensor(
                out=pt_t[:, 0:128], in0=pt_t[:, 0:128],
                in1=mask_sb[:, :], op=MULT)

        def emit_av(h, j, av, pt_t):
            for (cs, cw) in _av_chunks(j):
                nc.tensor.matmul(
                    av[:, cs:cs + cw],
                    lhsT=v_sb[j][:, h * 65:(h + 1) * 65],
                    rhs=pt_t[:, cs - j * 128:cs - j * 128 + cw],
                    start=(j == 0), stop=(j == SQT - 1),
                    skip_group_check=True)

        def emit_head_tail(h, av):
            # free the av PSUM slot fast: raw out^T + r row to SBUF,
            # then normalize off the critical path (no DRAM roundtrips).
            hp, hl = divmod(h, 2)
            raw = rpool.tile([64, S], BF16, tag="ats", name=f"ats{h}")
            nc.vector.tensor_copy(out=raw[:, :], in_=av[0:64, :])
            rt = rpool.tile([128, S], F32, tag="rt", name=f"rt{h}")
            nc.vector.tensor_copy(out=rt[64:65, :], in_=av[64:65, :])
            nc.gpsimd.dma_start(out=ri_dram.ap()[h:h + 1, :], in_=rt[64:65, :])
            nc.gpsimd.dma_start(
                out=rt[0:64, :],
                in_=bass.AP(tensor=ri_dram, offset=h * S, ap=[[0, 64], [1, S]]))
            nc.vector.reciprocal_approx_fast(out=rt[0:64, :], in_=rt[0:64, :])
            if hl == 0:
                nc.vector.tensor_tensor(
                    out=attT[hp][0:64, :], in0=raw[:, :], in1=rt[0:64, :], op=MULT)
            else:
                nc.vector.tensor_tensor(
                    out=raw[:, :], in0=raw[:, :], in1=rt[0:64, :], op=MULT)
                nc.sync.dma_start(out=attT[hp][64:128, :], in_=raw[:, :])

        def emit_head(h, scores_done=None):
            # interleaved: scores run one key-block ahead of AV
            av = avp.tile([65, S], F32, tag="av", name=f"av{h}")
            pts = scores_done
            if pts is None:
                pts = []
                for j in range(SQT):
                    pt_t = ptpool.tile([128, S - j * 128], BF16, tag=f"pt{j}",
                                       name=f"pt{j}_{h}")
                    pts.append(pt_t)
                    emit_scores(h, j, pt_t)
                    if j > 0:
                        emit_av(h, j - 1, av, pts[j - 1])
                emit_av(h, SQT - 1, av, pts[SQT - 1])
            else:
                for j in range(SQT):
                    emit_av(h, j, av, pts[j])
            emit_head_tail(h, av)

        # ---- emission order: keep ScalarE (exp) fed from t=~15us on ----
        emit_qk(0)
        h0_pts = []
        for j in range(SQT):
            pt_t = ptpool.tile([128, S - j * 128], BF16, tag=f"pt{j}",
                               name=f"pt{j}_h0")
            h0_pts.append(pt_t)
            emit_scores(0, j, pt_t)
        emit_v()
        emit_head(0, scores_done=h0_pts)
        emit_head(1)
        emit_qk(1)
        emit_head(2)
        emit_head(3)

        # ---- projection: outT[n, s] = sum_k wp[k, n] attT[k, s] ----
        for n in range(NX // 128):
            osb = opool.tile([128, 1024], F32, tag="osb", name=f"osb{n}")
            for sc in range(S // 512):
                ps = psp.tile([128, 512], F32, tag="sps")
                for kk in range(2):
                    nc.tensor.matmul(
                        ps[:, :],
                        lhsT=wp_sb[:, kk, n * 128:(n + 1) * 128],
                        rhs=attT[kk][:, sc * 512:(sc + 1) * 512],
                        start=(kk == 0), stop=(kk == 1))
                nc.vector.tensor_copy(
                    out=osb[:, (sc % 2) * 512:(sc % 2) * 512 + 512], in_=ps[:, :])
                if sc % 2 == 1:
                    nc.sync.dma_start(
                        out=outT.ap()[n * 128:(n + 1) * 128,
                                      (sc - 1) * 512:(sc + 1) * 512],
                        in_=osb[:, :])
                    if sc == 1:
                        osb = opool.tile([128, 1024], F32, tag="osb",
                                         name=f"osb{n}b")


def build_nc():
    import concourse.bass as bass
    import concourse.mybir as mybir
    import concourse.tile as tile
    from concourse import bacc
    dt = mybir.dt
    F32, BF16 = dt.float32, dt.bfloat16

    nc = bacc.Bacc("TRN2", target_bir_lowering=False, debug=False, num_devices=8)
    xT = nc.dram_tensor("xT", [NX, S], BF16, kind="ExternalInput")
    wq = nc.dram_tensor("wq", [NX, HPC * D], BF16, kind="ExternalInput")
    wk = nc.dram_tensor("wk", [NX, HPC * D], BF16, kind="ExternalInput")
    wv = nc.dram_tensor("wv", [NX, HPC * D], BF16, kind="ExternalInput")
    wp = nc.dram_tensor("wp", [HPC * D, NX], BF16, kind="ExternalInput")
    bq = nc.dram_tensor("bq", [HPC * D, 1], F32, kind="ExternalInput")
    bk = nc.dram_tensor("bk", [HPC * D, 1], F32, kind="ExternalInput")
    maskT = nc.dram_tensor("maskT", [128, 128], BF16, kind="ExternalInput")
    outT = nc.dram_tensor("outT", [NX, S], F32, kind="ExternalOutput")
    r_dram = nc.dram_tensor("r_scr", [HPC, S], F32)
    ri_dram = nc.dram_tensor("ri_scr", [HPC, S], F32)
    tens = (xT, wq, wk, wv, wp, bq, bk, maskT, outT, r_dram, ri_dram)

    with tile.TileContext(nc) as tc:
        _emit(nc, tc, bass, mybir, tens)
    nc.compile()
    return nc


def make_in_maps(x, w_attn, b_attn):
    bf = ml_dtypes.bfloat16
    maskT = np.triu(np.ones((128, 128), np.float32)).astype(bf)
    in_maps = []
    for core in range(8):
        b, g = divmod(core, 4)
        qs, ks, vs = 256 * g, NX + 256 * g, 2 * NX + 256 * g
        in_maps.append({
            "xT": np.ascontiguousarray(x[b].T).astype(bf),
            "wq": np.ascontiguousarray(w_attn[:, qs:qs + 256]).astype(bf),
            "wk": np.ascontiguousarray(w_attn[:, ks:ks + 256]).astype(bf),
            "wv": np.ascontiguousarray(w_attn[:, vs:vs + 256]).astype(bf),
            "wp": None,  # filled below (needs w_proj)
            "bq": b_attn[qs:qs + 256].reshape(256, 1).astype(np.float32),
            "bk": b_attn[ks:ks + 256].reshape(256, 1).astype(np.float32),
            "maskT": maskT,
        })
    return in_maps


def kernel(**inputs):
    global _nc_cache
    x = np.asarray(inputs["x"], np.float32)
    w_attn = np.asarray(inputs["w_attn"], np.float32)
    b_attn = np.asarray(inputs["b_attn"], np.float32)
    w_proj = np.asarray(inputs["w_proj"], np.float32)
    b_proj = np.asarray(inputs["b_proj"], np.float32)

    bf = ml_dtypes.bfloat16
    in_maps = make_in_maps(x, w_attn, b_attn)
    for core in range(8):
        g = core % 4
        in_maps[core]["wp"] = np.ascontiguousarray(
            w_proj[256 * g:256 * (g + 1), :]).astype(bf)

    if _nc_cache is None:
        _nc_cache = build_nc()
    from concourse.bass_utils import run_bass_kernel_spmd
    res = run_bass_kernel_spmd(_nc_cache, in_maps, core_ids=list(range(8)))

    out = np.zeros((B, S, NX), np.float32)
    for core in range(8):
        out[core // 4] += res.results[core]["outT"].T
    bv = b_attn[2 * NX:3 * NX]
    out += (bv @ w_proj + b_proj)[None, None, :]
    return out


# revision 7
# speedup vs baseline: 1.3241x; 1.0891x over previous
"""Distributed GPT-2 causal attention block for 8 TRN2 NeuronCores.

Sharding: data-parallel over batch (B=2) x tensor-parallel over heads
(16 heads -> 4 groups of 4). core = b*4 + g handles batch b, heads 4g..4g+3.

Per-core kernel (all compute in bf16, f32 PSUM accumulation):
  qT/kT = W[q|k]^T x^T        [2 tiles of 128 = 2 heads each, layout (h d) x S]
  v     = x W_v               [S x (4 heads x 65)], col 64 of each head = ones
  sT    = kT^T qT (transposed scores, row-packed 2 heads/matmul via tile rows)
  PT    = exp(sT/8) (ScalarE), causal: diag block masked multiplicatively,
          blocks above diag never computed
  av    = v_aug^T PT accumulated over key blocks -> [65 x S] PSUM;
          rows 0:64 unnormalized attn out^T, row 64 = softmax denominators r
  rinv  = 1/r via DRAM reshape roundtrip, broadcast to 64 partitions
  attT  = av[0:64] * rinv     [(h d) x S]
  outT  = wp^T attT           [NX x S] partial (sum over head groups on host)

Host: shard/cast inputs, run SPMD on cores 0-7, transpose+reduce partials,
fold b_attn's v-bias and b_proj in on the host (exact: softmax rows sum to 1).
"""

import numpy as np
import ml_dtypes

B, S, NX = 2, 2048, 1024
H, D = 16, 64
HPC = 4        # heads per core
KCH = NX // 128  # 8 contraction chunks
SQT = S // 128   # 16 query tiles
SCALE = 0.125    # 1/sqrt(D)

_nc_cache = None


def _av_chunks(j):
    """Bank-aligned (start, width) chunks of [j*128, S) on the 512-f32 grid."""
    out = []
    s = j * 128
    while s < S:
        e = min((s // 512 + 1) * 512, S)
        out.append((s, e - s))
        s = e
    return out


def _emit(nc, tc, bass, mybir, tens):
    dt = mybir.dt
    F32, BF16 = dt.float32, dt.bfloat16
    MULT = mybir.AluOpType.mult
    EXP = mybir.ActivationFunctionType.Exp
    xT, wq, wk, wv, wp, bq, bk, maskT, outT, r_dram, ri_dram = tens

    import contextlib
    with contextlib.ExitStack() as ctx:
        consts = ctx.enter_context(tc.tile_pool(name="consts", bufs=1))
        wpool = ctx.enter_context(tc.tile_pool(name="wpool", bufs=1))
        xpool = ctx.enter_context(tc.tile_pool(name="xpool", bufs=1))
        qkpool = ctx.enter_context(tc.tile_pool(name="qkpool", bufs=1))
        vpool = ctx.enter_context(tc.tile_pool(name="vpool", bufs=1))
        ptpool = ctx.enter_context(tc.tile_pool(name="ptpool", bufs=1))
        atpool = ctx.enter_context(tc.tile_pool(name="atpool", bufs=1))
        rpool = ctx.enter_context(tc.tile_pool(name="rpool", bufs=2))
        opool = ctx.enter_context(tc.tile_pool(name="opool", bufs=2))
        psp = ctx.enter_context(tc.tile_pool(name="psp", bufs=2, space="PSUM"))
        avp = ctx.enter_context(tc.tile_pool(name="avp", bufs=1, space="PSUM"))

        # ---- constants & weights ----
        mask_sb = consts.tile([128, 128], BF16, tag="mask")
        nc.sync.dma_start(out=mask_sb[:, :], in_=maskT.ap())
        bq_sb = consts.tile([128, 2], F32, tag="bq")
        nc.sync.dma_start(out=bq_sb[:, :],
                          in_=bq.ap().rearrange("(t p) o -> p (t o)", p=128))
        bk_sb = consts.tile([128, 2], F32, tag="bk")
        nc.sync.dma_start(out=bk_sb[:, :],
                          in_=bk.ap().rearrange("(t p) o -> p (t o)", p=128))

        wq_sb = wpool.tile([128, KCH, HPC * D], BF16, tag="wq")
        nc.sync.dma_start(out=wq_sb[:], in_=wq.ap().rearrange("(k p) n -> p k n", p=128))
        wk_sb = wpool.tile([128, KCH, HPC * D], BF16, tag="wk")
        nc.sync.dma_start(out=wk_sb[:], in_=wk.ap().rearrange("(k p) n -> p k n", p=128))
        wv_sb = wpool.tile([128, KCH, HPC * D], BF16, tag="wv")
        nc.sync.dma_start(out=wv_sb[:], in_=wv.ap().rearrange("(k p) n -> p k n", p=128))
        wp_sb = wpool.tile([128, 2, NX], BF16, tag="wp")
        nc.sync.dma_start(out=wp_sb[:], in_=wp.ap().rearrange("(k p) n -> p k n", p=128))

        x_sb = []
        for k in range(KCH):
            t = xpool.tile([128, S], BF16, tag=f"x{k}", name=f"x{k}")
            nc.sync.dma_start(out=t[:, :], in_=xT.ap()[k * 128:(k + 1) * 128, :])
            x_sb.append(t)

        qt_sb, kt_sb = [], []
        for t in range(2):
            qt_sb.append(qkpool.tile([128, S], BF16, tag=f"qt{t}", name=f"qt{t}"))
            kt_sb.append(qkpool.tile([128, S], BF16, tag=f"kt{t}", name=f"kt{t}"))
        v_sb = [vpool.tile([128, HPC * 65], BF16, tag=f"v{j}", name=f"v{j}")
                for j in range(SQT)]
        attT = [atpool.tile([128, S], BF16, tag=f"attT{hp}", name=f"attT{hp}")
                for hp in range(2)]

        def emit_qk_unit(t, u):
            # one 512-col chunk of qT (u<4) or kT (u>=4) for pair t
            (dst, wsb, bsb) = ((qt_sb, wq_sb, bq_sb), (kt_sb, wk_sb, bk_sb))[u // 4]
            c = u % 4
            ps = psp.tile([128, 512], F32, tag="sps")
            for k in range(KCH):
                nc.tensor.matmul(
                    ps[:, :],
                    lhsT=wsb[:, k, t * 128:(t + 1) * 128],
                    rhs=x_sb[k][:, c * 512:(c + 1) * 512],
                    start=(k == 0), stop=(k == KCH - 1))
            nc.vector.tensor_scalar_add(
                out=dst[t][:, c * 512:(c + 1) * 512], in0=ps[:, :],
                scalar1=bsb[:, t:t + 1])

        def emit_qk(t):
            for u in range(8):
                emit_qk_unit(t, u)

        def emit_v():
            for j in range(SQT):
                vt3 = v_sb[j].rearrange("p (h e) -> p h e", e=65)
                nc.gpsimd.memset(vt3[:, :, 64:65], 1.0)
                ps = psp.tile([128, 512], F32, tag="sps")
                for k in range(KCH):
                    nc.tensor.matmul(
                        ps[:, 0:HPC * D],
                        lhsT=x_sb[k][:, j * 128:(j + 1) * 128],
                        rhs=wv_sb[:, k, :],
                        start=(k == 0), stop=(k == KCH - 1))
                nc.vector.tensor_copy(
                    out=vt3[:, :, 0:64],
                    in_=ps[:, 0:HPC * D].rearrange("p (h d) -> p h d", d=64))

        def emit_scores(h, j, pt_t):
            # scores^T [key block j x queries j*128..S] -> exp -> PT, bf16
            hp, hl = divmod(h, 2)
            W = S - j * 128
            for c in range((W + 1023) // 1024):
                w = min(1024, W - c * 1024)
                o = j * 128 + c * 1024
                sps = psp.tile([128, 1024], F32, tag="sps")
                for cc in range((w + 511) // 512):
                    ww = min(512, w - cc * 512)
                    nc.tensor.matmul(
                        sps[:, cc * 512:cc * 512 + ww],
                        lhsT=kt_sb[hp][hl * 64:(hl + 1) * 64,
                                       j * 128:(j + 1) * 128],
                        rhs=qt_sb[hp][hl * 64:(hl + 1) * 64,
                                      o + cc * 512:o + cc * 512 + ww],
                        start=True, stop=True)
                nc.scalar.activation(
                    out=pt_t[:, c * 1024:c * 1024 + w],
                    in_=sps[:, :w], func=EXP, scale=SCALE)
            nc.vector.tensor_tensor(
                out=pt_t[:, 0:128], in0=pt_t[:, 0:128],
                in1=mask_sb[:, :], op=MULT)

        def emit_av(h, j, av, pt_t):
            for (cs, cw) in _av_chunks(j):
                nc.tensor.matmul(
                    av[:, cs:cs + cw],
                    lhsT=v_sb[j][:, h * 65:(h + 1) * 65],
                    rhs=pt_t[:, cs - j * 128:cs - j * 128 + cw],
                    start=(j == 0), stop=(j == SQT - 1),
                    skip_group_check=True)

        def emit_head_tail(h, av):
            # free the av PSUM slot fast: raw out^T + r row to SBUF,
            # then normalize off the critical path (no DRAM roundtrips).
            hp, hl = divmod(h, 2)
            raw = rpool.tile([64, S], BF16, tag="ats", name=f"ats{h}")
            nc.vector.tensor_copy(out=raw[:, :], in_=av[0:64, :])
            rt = rpool.tile([128, S], F32, tag="rt", name=f"rt{h}")
            nc.vector.tensor_copy(out=rt[64:65, :], in_=av[64:65, :])
            nc.gpsimd.dma_start(out=ri_dram.ap()[h:h + 1, :], in_=rt[64:65, :])
            nc.gpsimd.dma_start(
                out=rt[0:64, :],
                in_=bass.AP(tensor=ri_dram, offset=h * S, ap=[[0, 64], [1, S]]))
            nc.vector.reciprocal_approx_fast(out=rt[0:64, :], in_=rt[0:64, :])
            if hl == 0:
                nc.vector.tensor_tensor(
                    out=attT[hp][0:64, :], in0=raw[:, :], in1=rt[0:64, :], op=MULT)
            else:
                nc.vector.tensor_tensor(
                    out=raw[:, :], in0=raw[:, :], in1=rt[0:64, :], op=MULT)
                nc.sync.dma_start(out=attT[hp][64:128, :], in_=raw[:, :])

        # ---- software-pipelined heads: scores run one head ahead of AV ----
        ORDER = [0, 1, 3, 2]  # last head is an hl==0 head: tail needs no shift

        def emit_scores_head(i, h):
            tiles = []
            for j in range(SQT):
                pt_t = ptpool.tile([128, S - j * 128], BF16, tag=f"pt{i % 2}_{j}",
                                   name=f"pt{j}_h{h}")
                tiles.append(pt_t)
                emit_scores(h, j, pt_t)
            return tiles

        emit_qk(0)
        cur_pts = emit_scores_head(0, ORDER[0])
        emit_v()
        for i, h in enumerate(ORDER):
            av = avp.tile([65, S], F32, tag="av", name=f"av{h}")
            nxt_pts = []
            nxt = ORDER[i + 1] if i + 1 < len(ORDER) else None
            for j in range(SQT):
                if nxt is not None:
                    pt_t = ptpool.tile([128, S - j * 128], BF16,
                                       tag=f"pt{(i + 1) % 2}_{j}",
                                       name=f"pt{j}_h{nxt}")
                    nxt_pts.append(pt_t)
                    emit_scores(nxt, j, pt_t)
                if i == 0 and j < 8:
                    emit_qk_unit(1, j)
                emit_av(h, j, av, cur_pts[j])
            emit_head_tail(h, av)
            cur_pts = nxt_pts

        # ---- projection: outT[n, s] = sum_k wp[k, n] attT[k, s] ----
        for n in range(NX // 128):
            osb = opool.tile([128, 1024], BF16, tag="osb", name=f"osb{n}")
            for sc in range(S // 512):
                ps = psp.tile([128, 512], F32, tag="sps")
                for kk in range(2):
                    nc.tensor.matmul(
                        ps[:, :],
                        lhsT=wp_sb[:, kk, n * 128:(n + 1) * 128],
                        rhs=attT[kk][:, sc * 512:(sc + 1) * 512],
                        start=(kk == 0), stop=(kk == 1))
                nc.vector.tensor_copy(
                    out=osb[:, (sc % 2) * 512:(sc % 2) * 512 + 512], in_=ps[:, :])
                if sc % 2 == 1:
                    nc.sync.dma_start(
                        out=outT.ap()[n * 128:(n + 1) * 128,
                                      (sc - 1) * 512:(sc + 1) * 512],
                        in_=osb[:, :])
                    if sc == 1:
                        osb = opool.tile([128, 1024], BF16, tag="osb",
                                         name=f"osb{n}b")


def build_nc():
    import concourse.bass as bass
    import concourse.mybir as mybir
    import concourse.tile as tile
    from concourse import bacc
    dt = mybir.dt
    F32, BF16 = dt.float32, dt.bfloat16

    nc = bacc.Bacc("TRN2", target_bir_lowering=False, debug=False, num_devices=8)
    xT = nc.dram_tensor("xT", [NX, S], BF16, kind="ExternalInput")
    wq = nc.dram_tensor("wq", [NX, HPC * D], BF16, kind="ExternalInput")
    wk = nc.dram_tensor("wk", [NX, HPC * D], BF16, kind="ExternalInput")
    wv = nc.dram_tensor("wv", [NX, HPC * D], BF16, kind="ExternalInput")
    wp = nc.dram_tensor("wp", [HPC * D, NX], BF16, kind="ExternalInput")
    bq = nc.dram_tensor("bq", [HPC * D, 1], F32, kind="ExternalInput")
    bk = nc.dram_tensor("bk", [HPC * D, 1], F32, kind="ExternalInput")
    maskT = nc.dram_tensor("maskT", [128, 128], BF16, kind="ExternalInput")
    outT = nc.dram_tensor("outT", [NX, S], BF16, kind="ExternalOutput")
    r_dram = nc.dram_tensor("r_scr", [HPC, S], F32)
    ri_dram = nc.dram_tensor("ri_scr", [HPC, S], F32)
    tens = (xT, wq, wk, wv, wp, bq, bk, maskT, outT, r_dram, ri_dram)

    with tile.TileContext(nc) as tc:
        _emit(nc, tc, bass, mybir, tens)
    nc.compile()
    return nc


def make_in_maps(x, w_attn, b_attn):
    bf = ml_dtypes.bfloat16
    maskT = np.triu(np.ones((128, 128), np.float32)).astype(bf)
    in_maps = []
    for core in range(8):
        b, g = divmod(core, 4)
        qs, ks, vs = 256 * g, NX + 256 * g, 2 * NX + 256 * g
        in_maps.append({
            "xT": np.ascontiguousarray(x[b].T).astype(bf),
            "wq": np.ascontiguousarray(w_attn[:, qs:qs + 256]).astype(bf),
            "wk": np.ascontiguousarray(w_attn[:, ks:ks + 256]).astype(bf),
            "wv": np.ascontiguousarray(w_attn[:, vs:vs + 256]).astype(bf),
            "wp": None,  # filled below (needs w_proj)
            "bq": b_attn[qs:qs + 256].reshape(256, 1).astype(np.float32),
            "bk": b_attn[ks:ks + 256].reshape(256, 1).astype(np.float32),
            "maskT": maskT,
        })
    return in_maps


def kernel(**inputs):
    global _nc_cache
    x = np.asarray(inputs["x"], np.float32)
    w_attn = np.asarray(inputs["w_attn"], np.float32)
    b_attn = np.asarray(inputs["b_attn"], np.float32)
    w_proj = np.asarray(inputs["w_proj"], np.float32)
    b_proj = np.asarray(inputs["b_proj"], np.float32)

    bf = ml_dtypes.bfloat16
    in_maps = make_in_maps(x, w_attn, b_attn)
    for core in range(8):
        g = core % 4
        in_maps[core]["wp"] = np.ascontiguousarray(
            w_proj[256 * g:256 * (g + 1), :]).astype(bf)

    if _nc_cache is None:
        _nc_cache = build_nc()
    from concourse.bass_utils import run_bass_kernel_spmd
    res = run_bass_kernel_spmd(_nc_cache, in_maps, core_ids=list(range(8)))

    out = np.zeros((B, S, NX), np.float32)
    for core in range(8):
        out[core // 4] += res.results[core]["outT"].astype(np.float32).T
    bv = b_attn[2 * NX:3 * NX]
    out += (bv @ w_proj + b_proj)[None, None, :]
    return out
